# revision 5
# baseline (speedup 1.0000x reference)
"""BitNet b1.58 MLP (LLaMA-7B dims) on 8 Trainium2 NeuronCores.

Strategy: data-parallel over tokens (B*S=4096 -> 512 tokens/core), full
(replicated) ternary weights per core cast to bf16 (exact for ternary /
int8-range values). All three matmuls run in bf16 on the PE at full rate
with fp32 PSUM accumulation -- bit-exact integer results. The two global
absmean scales force two cross-core scalar AllReduces mid-kernel.

Per core pipeline:
  A: gate/up = x @ Wg^T, x @ Wu^T (per 128-row I-chunk), ga = silu(gate*gs),
     spill ga/up to HBM, accumulate sum|ga|  -> AllReduce #1 -> ga_s
  B: reload ga/up, ga_q = clip(rne(ga/ga_s)) via the +-1.5*2^23 magic-number
     trick, inter = ga_q*up*(ga_s*up_s), spill inter, accumulate sum|inter|
     -> AllReduce #2 -> inter_s
  C: inter_q = clip(rne(inter/inter_s)) -> bf16, out = inter_q @ Wd^T
Host gathers per-core token shards; no large collective needed.
"""

import numpy as np

B, S, H, I = 2, 2048, 4096, 11008
NCORES = 8
T = (B * S) // NCORES          # 512 tokens per core
ICH = I // 128                 # 86 I-chunks
HCH = H // 128                 # 32 H-chunks
NELEM = float(B * S * I)       # absmean denominator
EPS = 1e-8
INV_N = 1.0 / NELEM            # DVE has no divide; mult by f32 reciprocal
MAGIC = 12582912.0             # 1.5 * 2^23: (x + MAGIC) - MAGIC == rne(x)
QHI = MAGIC + 127.0
QLO = MAGIC - 128.0

_cached_nc = None


def _build():
    import concourse.tile as tile
    from concourse import mybir, bacc

    f32 = mybir.dt.float32
    bf16 = mybir.dt.bfloat16
    AX = mybir.AxisListType
    OP = mybir.AluOpType
    AF = mybir.ActivationFunctionType

    nc = bacc.Bacc("TRN2", target_bir_lowering=False, debug=False,
                   num_devices=NCORES)

    xt_in = nc.dram_tensor("xt", [HCH, 128, T], bf16, kind="ExternalInput")
    wg_in = nc.dram_tensor("wg", [ICH, HCH, 128, 128], bf16, kind="ExternalInput")
    wu_in = nc.dram_tensor("wu", [ICH, HCH, 128, 128], bf16, kind="ExternalInput")
    wd_in = nc.dram_tensor("wd", [HCH, ICH, 128, 128], bf16, kind="ExternalInput")
    sc_in = nc.dram_tensor("sc", [1, 4], f32, kind="ExternalInput")
    out_o = nc.dram_tensor("out", [HCH, 128, T], f32, kind="ExternalOutput")
    osc_o = nc.dram_tensor("oscale", [1, 1], f32, kind="ExternalOutput")

    with tile.TileContext(nc) as tc:
        with (
            tc.tile_pool(name="const", bufs=1) as const,
            tc.tile_pool(name="dram", bufs=1, space="DRAM") as dram,
            tc.tile_pool(name="psumS", bufs=1, space="PSUM") as psumS,
        ):
            # ---- constants / scalars ----
            sc_sb = const.tile([1, 4], f32, tag="sc_sb")
            nc.sync.dma_start(sc_sb[:], sc_in[:])
            sc_bc = const.tile([128, 4], f32, tag="sc_bc")
            nc.gpsimd.partition_broadcast(sc_bc[:], sc_sb[:])
            gate_s = sc_bc[:, 0:1]
            up_s = sc_bc[:, 1:2]
            wsd = sc_bc[:, 2:3]
            ones = const.tile([128, 1], f32, tag="ones")
            nc.vector.memset(ones[:], 1.0)
            stats1 = const.tile([128, ICH], f32, tag="stats1")
            stats2 = const.tile([128, ICH], f32, tag="stats2")

            # HBM scratch for the spilled intermediates
            ga_d = dram.tile([ICH, 128, T], f32)
            up_d = dram.tile([ICH, 128, T], f32)
            it_d = dram.tile([ICH, 128, T], f32)
            cc1_i = dram.tile([1, 1], f32)
            cc1_o = dram.tile([1, 1], f32)
            cc2_i = dram.tile([1, 1], f32)
            cc2_o = dram.tile([1, 1], f32)

            # ================= PHASE A =================
            with (
                tc.tile_pool(name="xpool", bufs=1) as xpool,
                tc.tile_pool(name="slab", bufs=3) as slab,
                tc.tile_pool(name="workA", bufs=3) as workA,
                tc.tile_pool(name="psumA", bufs=2, space="PSUM") as psumA,
            ):
                xt = xpool.tile([128, HCH, T], bf16, tag="xt")
                xsrc = xt_in[:].rearrange("hc h t -> h hc t")
                for piece in range(4):
                    lo = piece * 8
                    nc.sync.dma_start(xt[:, lo:lo + 8, :], xsrc[:, lo:lo + 8, :])

                for ic in range(ICH):
                    wg_t = slab.tile([128, HCH, 128], bf16, tag="wg")
                    wu_t = slab.tile([128, HCH, 128], bf16, tag="wu")
                    gsrc = wg_in[ic].rearrange("hc h i -> h hc i")
                    usrc = wu_in[ic].rearrange("hc h i -> h hc i")
                    for piece in range(4):
                        lo = piece * 8
                        nc.sync.dma_start(wg_t[:, lo:lo + 8, :], gsrc[:, lo:lo + 8, :])
                        nc.sync.dma_start(wu_t[:, lo:lo + 8, :], usrc[:, lo:lo + 8, :])

                    gps = psumA.tile([128, T], f32, tag="g")
                    ups = psumA.tile([128, T], f32, tag="u")
                    for hc in range(HCH):
                        nc.tensor.matmul(gps[:], wg_t[:, hc, :], xt[:, hc, :],
                                         start=(hc == 0), stop=(hc == HCH - 1))
                        nc.tensor.matmul(ups[:], wu_t[:, hc, :], xt[:, hc, :],
                                         start=(hc == 0), stop=(hc == HCH - 1))

                    ga_t = workA.tile([128, T], f32, tag="ga")
                    nc.scalar.activation(ga_t[:], gps[:], AF.Silu, scale=gate_s)
                    nc.vector.tensor_reduce(stats1[:, ic:ic + 1], ga_t[:],
                                            axis=AX.X, op=OP.add,
                                            apply_absolute_value=True)
                    up_t = workA.tile([128, T], f32, tag="up")
                    nc.scalar.copy(up_t[:], ups[:])
                    nc.sync.dma_start(ga_d[ic], ga_t[:])
                    nc.sync.dma_start(up_d[ic], up_t[:])

            # ---- AllReduce #1: global sum |ga| ----
            s1 = const.tile([128, 1], f32, tag="s1")
            nc.vector.tensor_reduce(s1[:], stats1[:], axis=AX.X, op=OP.add)
            ps1 = psumS.tile([1, 1], f32, tag="ps1")
            nc.tensor.matmul(ps1[:], s1[:], ones[:], start=True, stop=True)
            lsum1 = const.tile([1, 1], f32, tag="lsum1")
            nc.scalar.copy(lsum1[:], ps1[:])
            nc.sync.dma_start(cc1_i[:], lsum1[:])
            nc.gpsimd.collective_compute(
                "AllReduce", mybir.AluOpType.add,
                replica_groups=[list(range(NCORES))],
                ins=[cc1_i[:].opt()], outs=[cc1_o[:].opt()])
            gsum1 = const.tile([1, 1], f32, tag="gsum1")
            nc.sync.dma_start(gsum1[:], cc1_o[:])
            gsum1b = const.tile([128, 1], f32, tag="gsum1b")
            nc.gpsimd.partition_broadcast(gsum1b[:], gsum1[:])
            ga_sv = const.tile([128, 1], f32, tag="ga_sv")
            nc.vector.tensor_scalar(ga_sv[:], gsum1b[:], INV_N, EPS,
                                    op0=OP.mult, op1=OP.add)
            r_ga = const.tile([128, 1], f32, tag="r_ga")
            nc.vector.reciprocal(r_ga[:], ga_sv[:])
            s2 = const.tile([128, 1], f32, tag="s2")  # ga_s * up_s
            nc.vector.tensor_scalar(s2[:], ga_sv[:], up_s, None, op0=OP.mult)

            # ================= PHASE B =================
            with tc.tile_pool(name="workB", bufs=3) as workB:
                for ic in range(ICH):
                    ga_t = workB.tile([128, T], f32, tag="gaB")
                    up_t = workB.tile([128, T], f32, tag="upB")
                    nc.sync.dma_start(ga_t[:], ga_d[ic])
                    nc.sync.dma_start(up_t[:], up_d[ic])
                    t1 = workB.tile([128, T], f32, tag="t1")
                    # rne(ga/ga_s) via magic number: fma(ga, r, MAGIC)
                    nc.scalar.activation(t1[:], ga_t[:], AF.Copy,
                                         bias=MAGIC, scale=r_ga[:, :])
                    t2 = workB.tile([128, T], f32, tag="t2")
                    nc.vector.tensor_scalar(t2[:], t1[:], QHI, QLO,
                                            op0=OP.min, op1=OP.max)
                    gq = workB.tile([128, T], f32, tag="gq")
                    nc.vector.tensor_scalar(gq[:], t2[:], MAGIC, None,
                                            op0=OP.subtract)
                    ip = workB.tile([128, T], f32, tag="ip")
                    nc.vector.tensor_tensor(ip[:], gq[:], up_t[:], op=OP.mult)
                    it_t = workB.tile([128, T], f32, tag="it")
                    nc.scalar.activation(it_t[:], ip[:], AF.Copy, scale=s2[:, :])
                    nc.vector.tensor_reduce(stats2[:, ic:ic + 1], it_t[:],
                                            axis=AX.X, op=OP.add,
                                            apply_absolute_value=True)
                    nc.sync.dma_start(it_d[ic], it_t[:])

            # ---- AllReduce #2: global sum |inter| ----
            s2b = const.tile([128, 1], f32, tag="s2b")
            nc.vector.tensor_reduce(s2b[:], stats2[:], axis=AX.X, op=OP.add)
            ps2 = psumS.tile([1, 1], f32, tag="ps2")
            nc.tensor.matmul(ps2[:], s2b[:], ones[:], start=True, stop=True)
            lsum2 = const.tile([1, 1], f32, tag="lsum2")
            nc.scalar.copy(lsum2[:], ps2[:])
            nc.sync.dma_start(cc2_i[:], lsum2[:])
            nc.gpsimd.collective_compute(
                "AllReduce", mybir.AluOpType.add,
                replica_groups=[list(range(NCORES))],
                ins=[cc2_i[:].opt()], outs=[cc2_o[:].opt()])
            gsum2 = const.tile([1, 1], f32, tag="gsum2")
            nc.sync.dma_start(gsum2[:], cc2_o[:])
            gsum2b = const.tile([128, 1], f32, tag="gsum2b")
            nc.gpsimd.partition_broadcast(gsum2b[:], gsum2[:])
            it_sv = const.tile([128, 1], f32, tag="it_sv")
            nc.vector.tensor_scalar(it_sv[:], gsum2b[:], INV_N, EPS,
                                    op0=OP.mult, op1=OP.add)
            r_it = const.tile([128, 1], f32, tag="r_it")
            nc.vector.reciprocal(r_it[:], it_sv[:])
            osc_t = const.tile([128, 1], f32, tag="osc")
            nc.vector.tensor_scalar(osc_t[:], it_sv[:], wsd, None, op0=OP.mult)
            nc.sync.dma_start(osc_o[:], osc_t[0:1, :])

            # ================= PHASE C =================
            with (
                tc.tile_pool(name="iq", bufs=1) as iqpool,
                tc.tile_pool(name="workC", bufs=3) as workC,
                tc.tile_pool(name="wdslab", bufs=2) as wdslab,
                tc.tile_pool(name="outp", bufs=3) as outp,
                tc.tile_pool(name="psumC", bufs=4, space="PSUM") as psumC,
            ):
                iq_tiles = []
                for ic in range(ICH):
                    it_t = workC.tile([128, T], f32, tag="itC")
                    nc.sync.dma_start(it_t[:], it_d[ic])
                    c1 = workC.tile([128, T], f32, tag="c1")
                    nc.scalar.activation(c1[:], it_t[:], AF.Copy,
                                         bias=MAGIC, scale=r_it[:, :])
                    c2 = workC.tile([128, T], f32, tag="c2")
                    nc.vector.tensor_scalar(c2[:], c1[:], QHI, QLO,
                                            op0=OP.min, op1=OP.max)
                    iq = iqpool.tile([128, T], bf16, tag=f"iq{ic}")
                    nc.vector.tensor_scalar(iq[:], c2[:], MAGIC, None,
                                            op0=OP.subtract)
                    iq_tiles.append(iq)

                for hc in range(HCH):
                    wd_t = wdslab.tile([128, ICH, 128], bf16, tag="wd")
                    dsrc = wd_in[hc].rearrange("ic i h -> i ic h")
                    bounds = np.linspace(0, ICH, 9).astype(int)
                    for piece in range(8):
                        lo, hi = int(bounds[piece]), int(bounds[piece + 1])
                        nc.sync.dma_start(wd_t[:, lo:hi, :], dsrc[:, lo:hi, :])
                    ops = psumC.tile([128, T], f32, tag="o")
                    for ic in range(ICH):
                        nc.tensor.matmul(ops[:], wd_t[:, ic, :], iq_tiles[ic][:],
                                         start=(ic == 0), stop=(ic == ICH - 1))
                    ot = outp.tile([128, T], f32, tag="ot")
                    nc.scalar.copy(ot[:], ops[:])
                    nc.sync.dma_start(out_o[hc], ot[:])

    nc.compile()
    return nc


def _get_nc():
    global _cached_nc
    if _cached_nc is None:
        _cached_nc = _build()
    return _cached_nc


LAST_RESULT = None  # BassKernelResults of the most recent run (for profiling)


def kernel(x, x_scale, qw_gate, ws_gate, qw_up, ws_up, qw_down, ws_down,
           _profile=False):
    global LAST_RESULT
    import ml_dtypes
    from concourse.bass_utils import run_bass_kernel_spmd

    bf16 = ml_dtypes.bfloat16
    nc = _get_nc()

    x = np.asarray(x, dtype=np.float32)
    tokens = x.reshape(B * S, H)

    # weight slabs (shared across cores)
    wg = np.ascontiguousarray(
        np.asarray(qw_gate, np.float32).reshape(ICH, 128, HCH, 128)
        .transpose(0, 2, 3, 1)).astype(bf16)
    wu = np.ascontiguousarray(
        np.asarray(qw_up, np.float32).reshape(ICH, 128, HCH, 128)
        .transpose(0, 2, 3, 1)).astype(bf16)
    wd = np.ascontiguousarray(
        np.asarray(qw_down, np.float32).reshape(HCH, 128, ICH, 128)
        .transpose(0, 2, 3, 1)).astype(bf16)

    gate_s = np.float32(x_scale) * np.float32(ws_gate)
    up_s = np.float32(x_scale) * np.float32(ws_up)
    sc = np.array([[gate_s, up_s, np.float32(ws_down), 0.0]], np.float32)

    in_maps = []
    for c in range(NCORES):
        xt = np.ascontiguousarray(
            tokens[c * T:(c + 1) * T, :].T).astype(bf16).reshape(HCH, 128, T)
        in_maps.append({"xt": xt, "wg": wg, "wu": wu, "wd": wd, "sc": sc})

    res = run_bass_kernel_spmd(nc, in_maps, core_ids=list(range(NCORES)),
                               trace=_profile)
    LAST_RESULT = res

    out = np.empty((B * S, H), np.float32)
    for c in range(NCORES):
        oc = res.results[c]["out"]          # [HCH, 128, T]
        out[c * T:(c + 1) * T, :] = oc.reshape(H, T).T
    scale = np.float32(res.results[0]["oscale"][0, 0])
    return out.reshape(B, S, H), scale
